# revision 9
# baseline (speedup 1.0000x reference)
"""DD-RoPE kernel for 8x TRN2 NeuronCores.

Reference computation (B=4, T=4096, D=2048, P=256):
    deltas = einsum('btd,pd->btp', x, W) + b     # (B, T, P)
    angles = cumsum(deltas, axis=1)
    out = concat([x1*cos(a) - x2*sin(a), x2*cos(a) + x1*sin(a), x[..., 512:]], -1)

Sharding: 8 shards = 4 batches x 2 T-halves (2048 each), data-parallel.

Numerics: single fp16 matmul pass (W fp16, x fp16, fp32 PSUM). Host sends the
exact fp64 cumulative angle (from the TRUE weights) at every 64-step boundary,
so all quantization error is a <=64-step random walk (sim rel-err ~5e-3 vs
2e-2 budget). Bias + per-block bases enter through one extra "setup" matmul
per time block: stationary rows [b_hi, b_lo, (base_hi_k, base_lo_k)*] (bf16
pairs = near-exact f32 injection) against constant mover rows [1s, 1s,
one-hots]. The cumsum is then ONE masked segmented scan per [128, 512] block:
    state = (mask[t] * state) + dp[t]      (mask 0 at block starts)

Trig: range-reduce with the fp32 magic-round trick; sin = Sin(s*rs);
sneg = Sin(-s*rs); cos = Sin(-s*|rs| + s/4) (cos(2pi r) = sin(2pi(1/4-|r|))).
Rotation in fp16 2x-DVE mode with ADD-only combines (subtract would fall off
the DVE fast path, so o1 = x1*cos + x2*sneg).

Steady state per half-block: PE 17 matmuls (216ns cadence at 2.4GHz);
DVE scan+rs+|rs|+2 muls+2 adds; GpSimd 2 muls; Scalar 4 ACTs. Stages are
software-pipelined (manual skew) so in-order engine queues never stall on
same-iteration dependencies; the last round is column-split to halve the
drain latency. DMA (~11.6MB/core total, the near-binding resource): x on the
sync DGE queue, weights on scalar's, outputs alternate sync/gpsimd; all
tensors host-pre-tiled so every DMA is a dense block.
"""

import sys

if "/opt/trn_rl_repo" not in sys.path:
    sys.path.insert(0, "/opt/trn_rl_repo")

from contextlib import ExitStack

import ml_dtypes
import numpy as np

import concourse.bacc as bacc
import concourse.bass as bass
import concourse.mybir as mybir
import concourse.tile as tile
from concourse.bass_utils import run_bass_kernel_spmd

F32 = mybir.dt.float32
F16 = mybir.dt.float16
BF16 = mybir.dt.bfloat16
ADD = mybir.AluOpType.add
SUB = mybir.AluOpType.subtract
MULT = mybir.AluOpType.mult
MAX = mybir.AluOpType.max
IDENT = mybir.ActivationFunctionType.Identity
SIN = mybir.ActivationFunctionType.Sin

D = 2048          # input feature dim (contraction)
P = 256           # delta-pairs dim
ROT = 2 * P       # rotated columns (512)
TL = 2048         # time steps per shard
TB = 512          # time block (one PSUM bank at fp32)
SB = 64           # scan block (base injection granularity)
NS = TB // SB     # scan blocks per time block (8)
NT = TL // TB     # time blocks per shard (4)
NBK = TL // SB    # scan blocks per shard (32)
KC = D // 128     # contraction chunks (16)
NR = 2 + 2 * NS   # setup-matmul stationary rows (18)
N_CORES = 8

MAGIC = 12582912.0          # 1.5 * 2**23: fp32 round-to-int magic constant
SCALE_2PI = 6.28310         # slightly < 2*pi so Sin args stay inside [-pi, pi]
COS_BIAS = SCALE_2PI / 4.0  # cos(2pi y) = sin(SCALE*(1/4 - |wrap(y)|))
NP_BF16 = np.dtype(ml_dtypes.bfloat16)


def build_program(tl: int = TL) -> bass.Bass:
    nt = tl // TB
    nit = nt * 2          # half-block iterations (8)
    nc = bacc.Bacc("TRN2", target_bir_lowering=False, debug=False)

    # Host-pre-tiled inputs: every DMA below reads one dense DRAM block.
    # xf row block tb: [128, KC*TB] fp16 (d-chunks along the free dim)
    xf = nc.dram_tensor("xf", [nt * 128, KC * TB], F16,
                        kind="ExternalInput").ap()
    # weights, h-half major then d-chunks along the free dim
    wh = nc.dram_tensor("wh", [128, 2 * KC * 128], F16,
                        kind="ExternalInput").ap()
    # setup-matmul stationary rows per (tb, h): [NR, nit*128] bf16
    ext = nc.dram_tensor("ext", [NR, nit * 128], BF16,
                         kind="ExternalInput").ap()
    # constant mover rows: [NR, TB] bf16 (1s, 1s, one-hot pairs)
    mov = nc.dram_tensor("mov", [NR, TB], BF16, kind="ExternalInput").ap()
    # out row block tb: [128, 4*TB] fp16, h-major quadrants o1h0|o2h0|o1h1|o2h1
    outT = nc.dram_tensor("outT", [nt * 128, 4 * TB], F16,
                          kind="ExternalOutput").ap()

    with tile.TileContext(nc) as tc, ExitStack() as ctx:
        const_pool = ctx.enter_context(tc.tile_pool(name="const", bufs=1))
        x_pool = ctx.enter_context(tc.tile_pool(name="x", bufs=4))
        psum_pool = ctx.enter_context(tc.tile_pool(name="psum", bufs=4, space="PSUM"))
        ang_pool = ctx.enter_context(tc.tile_pool(name="ang", bufs=3))
        as_pool = ctx.enter_context(tc.tile_pool(name="as", bufs=3))
        rs_pool = ctx.enter_context(tc.tile_pool(name="rs", bufs=2))
        ab_pool = ctx.enter_context(tc.tile_pool(name="ab", bufs=2))
        trig_pool = ctx.enter_context(tc.tile_pool(name="trig", bufs=3))
        rot_pool = ctx.enter_context(tc.tile_pool(name="rot", bufs=2))
        out_pool = ctx.enter_context(tc.tile_pool(name="out", bufs=2))

        # Tiny ext/mov lead the sync queue (they gate the setup matmul);
        # wh halves lead the scalar queue (they gate the weight matmuls).
        ext_sb = const_pool.tile([NR, nit * 128], BF16, tag="ext")
        nc.sync.dma_start(ext_sb[:], ext[:])
        mov_sb = const_pool.tile([NR, TB], BF16, tag="mov")
        nc.sync.dma_start(mov_sb[:], mov[:])
        wh_sb = const_pool.tile([128, 2 * KC * 128], F16, tag="wh")
        hw_cols = KC * 128
        nc.scalar.dma_start(wh_sb[:, 0:hw_cols], wh[:, 0:hw_cols])
        nc.scalar.dma_start(wh_sb[:, hw_cols:2 * hw_cols],
                            wh[:, hw_cols:2 * hw_cols])
        # scan mask built in-place: 1s with 0 at every block-start column
        msk_sb = const_pool.tile([128, TB], F16, tag="msk")
        nc.gpsimd.memset(msk_sb[:], 1.0)
        nc.gpsimd.memset(msk_sb[:, 0:TB:SB], 0.0)
        magic_sb = const_pool.tile([128, 1], F32, tag="magic")
        nc.gpsimd.memset(magic_sb[:], MAGIC)
        cosb_sb = const_pool.tile([128, 1], F32, tag="cosb")
        nc.gpsimd.memset(cosb_sb[:], COS_BIAS)

        # ---- software-pipelined stages -------------------------------
        xall = {}     # per tb
        oall = {}     # per tb
        dp = {}       # per iteration i = tb*2 + h
        ang = {}
        a_s = {}
        rs = {}
        ab = {}
        sin16 = {}
        sneg16 = {}
        cos16 = {}

        def st_dma_in(tb):
            xall[tb] = x_pool.tile([128, KC * TB], F16, tag="xall", name="xall")
            nsub = 8 if tb == 0 else 4
            q = KC * TB // nsub
            for j in range(nsub):
                nc.sync.dma_start(xall[tb][:, j * q:(j + 1) * q],
                                  xf[tb * 128:(tb + 1) * 128, j * q:(j + 1) * q])

        def st_matmul(i):
            tb, h = i // 2, i % 2
            dpi = psum_pool.tile([128, TB], F32, tag="dp", name="dp")
            dp[i] = dpi
            nc.tensor.matmul(dpi[:], ext_sb[:, i * 128:(i + 1) * 128],
                             mov_sb[:], start=True, stop=False)
            for d in range(KC):
                ws = slice((h * KC + d) * 128, (h * KC + d + 1) * 128)
                xs = slice(d * TB, (d + 1) * TB)
                nc.tensor.matmul(dpi[:], wh_sb[:, ws], xall[tb][:, xs],
                                 start=False, stop=(d == KC - 1))

        def st_scan(i, parts=1):
            # masked segmented scan: state = mask[t]*state + dp[t].
            # Every SB boundary resets the state, so column splits at
            # SB multiples are exact (initial never propagates past them).
            ang[i] = ang_pool.tile([128, TB], F32, tag="ang", name="ang")
            a_s[i] = as_pool.tile([128, TB], F32, tag="a_s", name="a_s")
            w = TB // parts
            for p in range(parts):
                cs = slice(p * w, (p + 1) * w)
                nc.vector.tensor_tensor_scan(ang[i][:, cs], msk_sb[:, cs],
                                             dp[i][:, cs], initial=0.0,
                                             op0=MULT, op1=ADD)
                # a_s = MAGIC - round(ang)  (fp32 magic rounding)
                nc.scalar.activation(a_s[i][:, cs], ang[i][:, cs], IDENT,
                                     bias=magic_sb[:], scale=-1.0)
            del dp[i]

        def st_trig(i, parts=1):
            # rs = ang - round(ang) in [-0.5, 0.5]; ab = |rs| (DVE max-trick)
            rs[i] = rs_pool.tile([128, TB], F32, tag="rs", name="rs")
            ab[i] = ab_pool.tile([128, TB], F32, tag="ab", name="ab")
            sin16[i] = trig_pool.tile([128, TB], F16, tag="sin", name="sin16")
            sneg16[i] = trig_pool.tile([128, TB], F16, tag="sng", name="sneg16")
            cos16[i] = trig_pool.tile([128, TB], F16, tag="cos", name="cos16")
            w = TB // parts
            for p in range(parts):
                cs = slice(p * w, (p + 1) * w)
                nc.vector.scalar_tensor_tensor(rs[i][:, cs], a_s[i][:, cs],
                                               MAGIC, ang[i][:, cs],
                                               op0=SUB, op1=ADD)
                nc.vector.scalar_tensor_tensor(ab[i][:, cs], rs[i][:, cs],
                                               -1.0, rs[i][:, cs],
                                               op0=MULT, op1=MAX)
                nc.scalar.activation(sin16[i][:, cs], rs[i][:, cs], SIN,
                                     scale=SCALE_2PI)
                nc.scalar.activation(sneg16[i][:, cs], rs[i][:, cs], SIN,
                                     scale=-SCALE_2PI)
                nc.scalar.activation(cos16[i][:, cs], ab[i][:, cs], SIN,
                                     scale=-SCALE_2PI, bias=cosb_sb[:])
            del a_s[i], ang[i], rs[i], ab[i]

        def st_rot(i, parts=1):
            tb, h = i // 2, i % 2
            if h == 0:
                oall[tb] = out_pool.tile([128, 4 * TB], F16, tag="oall",
                                         name="oall")
            x1s = xall[tb][:, h * TB:(h + 1) * TB]
            x2s = xall[tb][:, (2 + h) * TB:(3 + h) * TB]
            st, sn, ct = sin16[i], sneg16[i], cos16[i]
            t1 = rot_pool.tile([128, TB], F16, tag="t1")
            t2 = rot_pool.tile([128, TB], F16, tag="t2")
            t3 = rot_pool.tile([128, TB], F16, tag="t3")
            t4 = rot_pool.tile([128, TB], F16, tag="t4")
            o1 = oall[tb][:, 2 * h * TB:(2 * h + 1) * TB]
            o2 = oall[tb][:, (2 * h + 1) * TB:(2 * h + 2) * TB]
            w = TB // parts
            for p in range(parts):
                cs = slice(p * w, (p + 1) * w)
                nc.vector.tensor_mul(t1[:, cs], x1s[:, cs], ct[:, cs])
                nc.gpsimd.tensor_mul(t2[:, cs], x2s[:, cs], sn[:, cs])
                nc.gpsimd.tensor_mul(t3[:, cs], x2s[:, cs], ct[:, cs])
                nc.vector.tensor_mul(t4[:, cs], x1s[:, cs], st[:, cs])
                # o1 = x1*cos + x2*(-sin); o2 = x2*cos + x1*sin
                nc.vector.tensor_add(o1[:, cs], t1[:, cs], t2[:, cs])
                nc.vector.tensor_add(o2[:, cs], t3[:, cs], t4[:, cs])
            del sin16[i], sneg16[i], cos16[i]
            rows = slice(tb * 128, (tb + 1) * 128)
            ocs = slice(2 * h * TB, 2 * (h + 1) * TB)
            if i == nit - 1:
                hcs1 = slice(2 * h * TB, (2 * h + 1) * TB)
                hcs2 = slice((2 * h + 1) * TB, 2 * (h + 1) * TB)
                nc.sync.dma_start(outT[rows, hcs1], oall[tb][:, hcs1])
                nc.gpsimd.dma_start(outT[rows, hcs2], oall[tb][:, hcs2])
            else:
                dma_eng = nc.gpsimd if i % 2 == 0 else nc.sync
                dma_eng.dma_start(outT[rows, ocs], oall[tb][:, ocs])
            if h == 1:
                del xall[tb], oall[tb]

        # prime the x DMA pipeline two blocks deep, then run skewed stages;
        # matmuls are emitted last in each round so cross-engine semaphore
        # thresholds reference the oldest possible PE queue positions
        st_dma_in(0)
        st_dma_in(1)
        for r in range(nit + 3):
            last = nit - 1
            if 0 <= r - 1 < nit:
                st_scan(r - 1, parts=2 if r - 1 == last else 1)
            if 0 <= r - 2 < nit:
                st_trig(r - 2, parts=2 if r - 2 == last else 1)
            if 0 <= r - 3 < nit:
                st_rot(r - 3, parts=2 if r - 3 == last else 1)
            if r < nit:
                if r % 2 == 1 and r // 2 + 2 < nt:
                    st_dma_in(r // 2 + 2)
                st_matmul(r)

    nc.compile()
    return nc


_NC_CACHE: dict = {}


def _get_nc():
    if "nc" not in _NC_CACHE:
        _NC_CACHE["nc"] = build_program()
    return _NC_CACHE["nc"]


def _tile_x(xt16: np.ndarray, nt: int) -> np.ndarray:
    """[D, tl] fp16 -> [nt*128, KC*TB]: row block tb, d-chunks along free."""
    tl = xt16.shape[1]
    a = xt16.reshape(KC, 128, tl // TB, TB).transpose(2, 1, 0, 3)
    return np.ascontiguousarray(a.reshape((tl // TB) * 128, KC * TB))


def _split_bf16(v: np.ndarray):
    hi = v.astype(NP_BF16)
    lo = (v - hi.astype(np.float64)).astype(NP_BF16)
    return hi, lo


def prepare_weights(W: np.ndarray, b: np.ndarray):
    inv2pi = 1.0 / (2.0 * np.pi)
    Wt = W.astype(np.float64).T * inv2pi                           # [D, P]
    bt = b.astype(np.float64) * inv2pi                             # [P]
    whf = Wt.astype(np.float16)
    # [D, P] -> [128, 2*KC*128]: h-half major, then d-chunks along free dim
    wh_in = np.ascontiguousarray(
        whf.reshape(KC, 128, 2, 128).transpose(1, 2, 0, 3).reshape(128, 2 * KC * 128))
    return wh_in, Wt, bt


def make_in_maps(x: np.ndarray, W: np.ndarray, b: np.ndarray):
    B = x.shape[0]
    wh_in, Wt, bt = prepare_weights(W, b)

    # fp64 cumulative angle at every SB-step boundary, per batch (in turns),
    # computed from the TRUE weights so the W-quantization error is also a
    # <=SB-step random walk. Wrapped mod 1 to keep scan values small.
    T = x.shape[1]
    nblk = T // SB
    xblk = x.reshape(B, nblk, SB, D).sum(axis=2, dtype=np.float64)  # [B, nblk, D]
    dblk = xblk @ Wt + SB * bt                                      # [B, nblk, P]
    bases = np.zeros((B, nblk, P))
    np.cumsum(dblk[:, :-1], axis=1, out=bases[:, 1:])               # exclusive
    bases -= np.round(bases)

    b_hi, b_lo = _split_bf16(bt)

    # constant mover rows [NR, TB]: 1s, 1s, then one-hot pairs at k*SB
    mov_in = np.zeros((NR, TB), NP_BF16)
    mov_in[0] = 1.0
    mov_in[1] = 1.0
    for k in range(NS):
        mov_in[2 + 2 * k, k * SB] = 1.0
        mov_in[3 + 2 * k, k * SB] = 1.0

    in_maps = []
    for c in range(N_CORES):
        bb, hh = c // 2, c % 2
        xt16 = x[bb, hh * TL:(hh + 1) * TL, :].T.astype(np.float16)
        bs = bases[bb, hh * NBK:(hh + 1) * NBK]                     # [NBK, P]
        # ext rows per (tb, h): [NR, nit*128]
        ext_in = np.zeros((NR, NT * 2 * 128), NP_BF16)
        for tb in range(NT):
            for h in range(2):
                i = tb * 2 + h
                cols = slice(i * 128, (i + 1) * 128)
                ps = slice(h * 128, (h + 1) * 128)
                ext_in[0, cols] = b_hi[ps]
                ext_in[1, cols] = b_lo[ps]
                for k in range(NS):
                    bhi, blo = _split_bf16(bs[tb * NS + k, ps])
                    ext_in[2 + 2 * k, cols] = bhi
                    ext_in[3 + 2 * k, cols] = blo
        in_maps.append({
            "xf": _tile_x(xt16, NT),
            "wh": wh_in,
            "ext": np.ascontiguousarray(ext_in),
            "mov": mov_in,
        })
    return in_maps


def assemble_output(x: np.ndarray, results) -> np.ndarray:
    B, T, Din = x.shape
    out = np.empty((B, T, Din), np.float32)
    out[:, :, ROT:] = x[:, :, ROT:]
    for c in range(N_CORES):
        bb, hh = c // 2, c % 2
        # quadrants are h-major (o1h0|o2h0|o1h1|o2h1): q = h*2 + oi
        r = results[c]["outT"].astype(np.float32).reshape(NT, 128, 2, 2, TB)
        # [tb, pp, h, oi, u] -> [t_local(tb,u), p(oi,h,pp)]
        blk = r.transpose(0, 4, 3, 2, 1).reshape(TL, ROT)
        out[bb, hh * TL:(hh + 1) * TL, :ROT] = blk
    return out


def kernel(x: np.ndarray, W: np.ndarray, b: np.ndarray) -> np.ndarray:
    nc = _get_nc()
    in_maps = make_in_maps(x, W, b)
    res = run_bass_kernel_spmd(nc, in_maps, list(range(N_CORES)))
    return assemble_output(x, res.results)
